# revision 43
# baseline (speedup 1.0000x reference)
"""Sparse (causal + noncausal-prefix) attention on 8 TRN2 NeuronCores.

Sharding: 2 batches x 16 heads = 32 (b,h) pairs, 2 heads per core
(head-parallel). Each core receives the full x (pre-transposed to
[128, chunk, tokens] bf16 on host), its 2 heads' slices of w_qkv, and
its 128 rows of w_out; it computes QKV projection, causal attention
(scores kept transposed: [keys, queries], softmax without
max-subtraction -- inputs are N(0,~0.4) so exp never overflows;
row-sums obtained by appending a ones-column to V in the PV matmul),
and a row-sharded output projection. Host sums the 8 partial outputs
and adds b_out.

Key scheduling choices (all measured on HW traces):
- Input DMAs are a few large-line transfers; bulk loads ride the sync
  HWDGE ring (descriptor generation occupies the issuing engine, and
  the scalar engine must stay free for exp), weights ride the scalar
  ring early. The first 512 tokens arrive as 2-chunk pieces matched
  by chunk-outer QKV matmul order so compute starts ~9us in.
- Both heads' scores live in one [128,2,512] 2-bank PSUM tile so a
  single ACT exp instruction covers both heads (ACT is the attention
  inner-loop pacer at ~1us/key-block).
- V is padded to 128 columns: LDWEIGHTS only gets fast-weight-load
  with exactly 128 columns.
- Tasks run b0q0..b0q3, b1q1, b1q2, b1q3, b1q0 so the last task is
  small; the final two projection groups use (by then free) score
  PSUM slots and drain half-blocks over both DMA rings as soon as
  each copy lands.
"""
import numpy as np
import ml_dtypes

import concourse.bass as bass
import concourse.tile as tile
from concourse import bacc, mybir
from concourse.bass_utils import run_bass_kernel_spmd

# Problem constants (hardcoded per contract).
B = 2
N = 2048
D = 1024
HEADS = 16
DH = 64
NONCAUSAL = 64
T = B * N          # 4096 tokens
N_CORES = 8
HPC = HEADS // N_CORES   # heads per core = 2
IPC = HPC * DH           # inner dims per core = 128
NKC = D // 128           # contraction chunks = 8
TB = T // 128            # 128-token blocks = 32
QSB = 512                # query superblock
NQSB = N // QSB          # 4 per batch

BF16 = mybir.dt.bfloat16
F32 = mybir.dt.float32
NP_BF16 = ml_dtypes.bfloat16

# Task order: (b, qsb) pairs; last task is the smallest (4 key blocks)
# so the serial tail after the final exp is short.
TASK_ORDER = [(0, 0), (0, 1), (0, 2), (0, 3), (1, 1), (1, 2), (1, 3), (1, 0)]
# stage1 jobs (512-token QKV superblocks, 0..7 over both batches)
# emitted at the start of each task; each job must precede any task
# whose keys or queries touch its tokens.
STAGE1_AT_TASK = [[0], [1], [2, 4], [3, 5], [6], [7], [], []]
# projection groups: list of (task_pos_emitted_after_loop). Tasks are
# identified by their position in TASK_ORDER.
PROJ_AFTER_LOOP = {2: 0, 3: 1, 4: 2, 5: 3, 6: 5}
# pos -> list of (inject_kb, proj task_pos)
PROJ_INJECT = {6: [(6, 4)], 7: [(3, 6)]}
# Incremental per-diagonal-block tail drain (measured slower: the
# inline bcast/proj matmuls head-of-line block the PE stream): off.
TAILPIPE_POS = set()

_CACHED_NC = None


def build_nc():
    global _CACHED_NC
    if _CACHED_NC is not None:
        return _CACHED_NC
    nc = bacc.Bacc("TRN2", target_bir_lowering=False, debug=False)

    # host-prepacked layouts (see make_in_maps)
    xt = nc.dram_tensor("xt", [128, NKC, T], BF16, kind="ExternalInput").ap()
    wqkv = nc.dram_tensor("wqkv", [128, 3, NKC, IPC], BF16,
                          kind="ExternalInput").ap()
    wo = nc.dram_tensor("wo", [IPC, D], BF16, kind="ExternalInput").ap()
    mpack = nc.dram_tensor("mpack", [128, 320], BF16, kind="ExternalInput").ap()
    out = nc.dram_tensor("out", [T, D], BF16, kind="ExternalOutput").ap()

    with tile.TileContext(nc) as tc:
        with (
            tc.tile_pool(name="xt", bufs=1) as xt_pool,
            tc.tile_pool(name="w", bufs=1) as w_pool,
            tc.tile_pool(name="qkv", bufs=1) as qkv_pool,
            tc.tile_pool(name="ex", bufs=16) as ex_pool,
            tc.tile_pool(name="small", bufs=4) as small_pool,
            tc.tile_pool(name="raw", bufs=4) as raw_pool,
            # 8 bufs: at the tail, consecutive projection groups rotate
            # through these while the out-DMA ring is backlogged; 4 bufs
            # made the final copies wait on earlier blocks' DMA drain.
            tc.tile_pool(name="osb", bufs=8) as osb_pool,
            # scores: both heads' [128,512] f32 in ONE 2-bank tile so a
            # single ACT exp instruction covers both heads (halves the
            # per-instruction ACT overhead in the attention inner loop).
            tc.tile_pool(name="ps_sc", bufs=2, space="PSUM") as ps_sc,
            tc.tile_pool(name="ps_io", bufs=2, space="PSUM") as ps_io,
            tc.tile_pool(name="ps_out", bufs=2, space="PSUM") as ps_out,
        ):
            # ---- SBUF tiles ----
            xt_sb = xt_pool.tile([128, NKC, T], BF16, tag="xt")
            w_sb = w_pool.tile([128, 3, NKC, IPC], BF16, tag="w")
            wo_sb = w_pool.tile([IPC, D], BF16, tag="wo")
            mp_sb = w_pool.tile([128, 320], BF16, tag="mp")
            wq_sb = w_sb[:, 0]
            wk_sb = w_sb[:, 1]
            wv_sb = w_sb[:, 2]
            tri_sb = mp_sb[:, 0:128]
            trinc_sb = mp_sb[:, 128:256]
            ones_sb = mp_sb[0:1, 256:256 + DH]

            # ---- input DMAs ----
            # All bulk loads go on the sync HWDGE ring: descriptor
            # generation (~4.4ns/desc) occupies the issuing engine, and
            # the scalar engine must stay free for exp. Weights ride the
            # scalar ring early (small, before exps exist). The first
            # 512 tokens arrive as 2-chunk pieces so QKV compute starts
            # as soon as the first chunks land (chunk-outer matmul
            # order below matches arrival order).
            for cc in range(0, NKC, 2):
                nc.sync.dma_start(xt_sb[:, cc:cc + 2, 0:512],
                                  xt[:, cc:cc + 2, 0:512])
            # wq first (gates the very first matmul), then wk|wv
            nc.scalar.dma_start(w_sb[:, 0:1], wqkv[:, 0:1])
            nc.scalar.dma_start(w_sb[:, 1:3], wqkv[:, 1:3])
            nc.sync.dma_start(xt_sb[:, :, 512:1024], xt[:, :, 512:1024])
            nc.scalar.dma_start(wo_sb[:], wo[:])
            nc.scalar.dma_start(mp_sb[:], mpack[:])
            nc.sync.dma_start(xt_sb[:, :, 1024:2048], xt[:, :, 1024:2048])
            nc.sync.dma_start(xt_sb[:, :, 2048:3072], xt[:, :, 2048:3072])
            nc.sync.dma_start(xt_sb[:, :, 3072:4096], xt[:, :, 3072:4096])

            qt_sb = qkv_pool.tile([IPC, T], BF16, tag="qt")
            kt_sb = qkv_pool.tile([IPC, T], BF16, tag="kt")
            # V padded to 128 columns: LDWEIGHTS only gets FWL (2x load
            # rate) when the weight has exactly 128 columns. Columns
            # DH+1..127 stay zero; column DH is the rowsum ones-column.
            v_sb = qkv_pool.tile([128, HPC, TB, 128], BF16, tag="v")
            attnt_sb = qkv_pool.tile([IPC, T], BF16, tag="attnt")

            nc.vector.memset(v_sb[:, :, :, DH + 1:128], 0.0)
            nc.vector.memset(v_sb[:, :, :, DH:DH + 1], 1.0)

            # ---- stage 1 (emitted per 512-token superblock, interleaved
            # with stage-2 tasks below so exp starts early) ----
            def emit_stage1(jt):
                tsl = bass.ts(jt, 512)
                # chunk-outer so the first job's matmuls start as soon
                # as the first 2-chunk DMA piece lands.
                pss = [ps_io.tile([128, 512], F32, tag="io", name="ps")
                       for _ in range(2)]
                for c in range(NKC):
                    for di, wx_sb in enumerate((wq_sb, wk_sb)):
                        nc.tensor.matmul(
                            pss[di][:],
                            wx_sb[:, c, :],
                            xt_sb[:, c, tsl],
                            start=(c == 0),
                            stop=(c == NKC - 1),
                        )
                for di, dst_sb in enumerate((qt_sb, kt_sb)):
                    nc.vector.tensor_copy(dst_sb[:, tsl], pss[di][:])
                for tb in range(jt * 4, jt * 4 + 4):
                    bsl = bass.ts(tb, 128)
                    psv = ps_out.tile([128, 512], F32, tag="out", name="psv")
                    for c in range(NKC):
                        nc.tensor.matmul(
                            psv[:, 0:IPC],
                            xt_sb[:, c, bsl],
                            wv_sb[:, c, :],
                            start=(c == 0),
                            stop=(c == NKC - 1),
                        )
                    nc.vector.tensor_copy(
                        v_sb[:, :, tb, 0:DH],
                        psv[:, 0:IPC].rearrange("p (h d) -> p h d", h=HPC),
                    )

            done = []

            def emit_proj(task, final=False):
                tb0 = task[0] * 16 + task[1] * 4
                for i, tb in enumerate(range(tb0, tb0 + 4)):
                    osb = osb_pool.tile([128, D], BF16, tag="osb", name="osb")
                    if final:
                        # drain-time projection: scores are done, so use
                        # the 2-bank score slots — one slot per tb lets
                        # consecutive tbs pipeline instead of chaining
                        # through the copy of the shared ps_io slot.
                        pr2 = ps_sc.tile([128, 2, 512], F32, tag="sc",
                                         name="pr2")
                        prs = (pr2[:, 0], pr2[:, 1])
                    else:
                        prs = tuple(
                            ps_io.tile([128, 512], F32, tag="io", name="pr")
                            for _ in range(2))
                    for half in range(2):
                        pr = prs[half]
                        nc.tensor.matmul(
                            pr,
                            attnt_sb[:, bass.ts(tb, 128)],
                            wo_sb[:, bass.ts(half, 512)],
                        )
                        if half == 0:
                            nc.vector.tensor_copy(osb[:, 0:512], pr)
                        else:
                            nc.scalar.copy(osb[:, 512:1024], pr)
                        if final:
                            # drain each half as soon as its copy lands,
                            # split across both DMA rings.
                            eng = nc.sync if half == 0 else nc.scalar
                            eng.dma_start(
                                out[bass.ts(tb, 128),
                                    half * 512:(half + 1) * 512],
                                osb[:, half * 512:(half + 1) * 512])
                    if not final:
                        nc.sync.dma_start(out[bass.ts(tb, 128), :], osb[:])

            def norm_cols(out_pss, q0, i0, ncols):
                # raw / rowsum (rowsum in row DH) for query columns
                # [i0, i0+ncols): broadcast rowsum via PE,
                # fast-reciprocal, multiply.
                csl = slice(i0, i0 + ncols)
                for h in range(HPC):
                    hsl = slice(h * DH, (h + 1) * DH)
                    out_ps = out_pss[h]
                    rs = small_pool.tile([1, 512], BF16, tag="rs",
                                         name="rs")
                    nc.vector.tensor_copy(rs[0:1, 0:ncols],
                                          out_ps[DH:DH + 1, csl])
                    bct = ps_io.tile([128, 512], F32, tag="io", name="bc")
                    bc = bct[0:DH, 0:ncols]
                    nc.tensor.matmul(bc, ones_sb, rs[0:1, 0:ncols])
                    rec = raw_pool.tile([DH, 512], F32, tag="rec64",
                                        name="rec")
                    nc.vector.reciprocal_approx_fast(rec[0:DH, 0:ncols], bc)
                    nc.vector.tensor_mul(
                        attnt_sb[hsl, q0 + i0:q0 + i0 + ncols],
                        out_ps[0:DH, csl], rec[0:DH, 0:ncols])

            def tail_block(i, out_pss, q0, tb0, b, qsb):
                # incremental drain for the last tasks: normalize column
                # block i, project its token block, DMA it out.
                norm_cols(out_pss, q0, i * 128, 128)
                tb = tb0 + i
                osb = osb_pool.tile([128, D], BF16, tag="osb", name="osb")
                pr2 = ps_sc.tile([128, 2, 512], F32, tag="sc", name="pr2")
                for half in range(2):
                    nc.tensor.matmul(
                        pr2[:, half],
                        attnt_sb[:, bass.ts(tb, 128)],
                        wo_sb[:, bass.ts(half, 512)],
                    )
                nc.vector.tensor_copy(osb[:, 0:512], pr2[:, 0])
                nc.scalar.copy(osb[:, 512:1024], pr2[:, 1])
                eng = nc.scalar if i % 2 == 1 else nc.sync
                eng.dma_start(out[bass.ts(tb, 128), :], osb[:])

            # ---- stage 2: attention per (batch, qsb), both heads fused ----
            pending_tail = None  # previous task's deferred tail closures
            for pos, (b, qsb) in enumerate(TASK_ORDER):
                for jt in STAGE1_AT_TASK[pos]:
                    emit_stage1(jt)
                n_kb = 4 * (qsb + 1)
                q0 = b * N + qsb * QSB
                tb0 = b * 16 + qsb * 4
                tailpipe = pos in TAILPIPE_POS
                out_pss = [ps_out.tile([128, 512], F32, tag="out",
                                       name="out_ps")
                           for _ in range(HPC)]
                pv_queue = []

                def emit_pv(kb, ex2_kb, out_pss=out_pss, n_kb=n_kb, b=b,
                            qsb=qsb, q0=q0, tb0=tb0, tailpipe=tailpipe):
                    j0 = max(0, (kb - 4 * qsb) * 128)
                    for h in range(HPC):
                        nc.tensor.matmul(
                            out_pss[h][:, j0:QSB],
                            v_sb[:, h, b * 16 + kb, :],
                            ex2_kb[:, h, j0:QSB],
                            start=(kb == 0),
                            stop=(kb == n_kb - 1),
                        )
                    if tailpipe and kb >= 4 * qsb:
                        tail_block(kb - 4 * qsb, out_pss, q0, tb0, b, qsb)

                for kb in range(n_kb):
                    j = kb - 4 * qsb
                    jq = max(0, 128 * j)  # masked leading queries
                    k0 = b * N + kb * 128
                    sc2 = ps_sc.tile([128, HPC, 512], F32, tag="sc",
                                     name="sc2")
                    ex2 = ex_pool.tile([128, HPC, 512], BF16, tag="ex",
                                       name="ex2")
                    # h0/h1 use disjoint PE row groups -> concurrent;
                    # emit back-to-back with no allocs in between.
                    for h in range(HPC):
                        hsl = slice(h * DH, (h + 1) * DH)
                        nc.tensor.matmul(
                            sc2[:, h, jq:QSB],
                            kt_sb[hsl, k0:k0 + 128],
                            qt_sb[hsl, q0 + jq:q0 + QSB],
                        )
                    # one exp covers both heads (2-bank PSUM read)
                    nc.scalar.activation(
                        ex2[:, :, jq:QSB] if j >= 0 else ex2[:],
                        sc2[:, :, jq:QSB] if j >= 0 else sc2[:],
                        mybir.ActivationFunctionType.Exp)
                    if j >= 0:
                        m_sb = trinc_sb if (qsb == 0 and kb == 0) else tri_sb
                        for h in range(HPC):
                            nc.gpsimd.tensor_mul(
                                ex2[:, h, jq:jq + 128],
                                ex2[:, h, jq:jq + 128],
                                m_sb,
                            )
                    # inject the previous task's tail here so its PE ops
                    # (flush PVs, bcast matmuls) never head-of-line block
                    # this task's scores at the boundary.
                    if pending_tail is not None and kb == 1:
                        pending_tail[0]()
                    if pending_tail is not None and kb == 2:
                        pending_tail[1]()
                        pending_tail = None
                    for inj_kb, inj_pos in PROJ_INJECT.get(pos, ()):
                        if kb == inj_kb:
                            emit_proj(TASK_ORDER[inj_pos],
                                      final=(pos == len(TASK_ORDER) - 1))
                    # software pipeline: PV trails scores by 2 kb so the
                    # in-order PE stream never waits on a fresh exp.
                    pv_queue.append((kb, ex2))
                    if len(pv_queue) > 2:
                        emit_pv(*pv_queue.pop(0))

                rem = list(pv_queue)
                if tailpipe:
                    # each remaining PV carries its own norm/proj/DMA
                    tails = (lambda r=rem[0], f=emit_pv: f(*r),
                             lambda r=rem[1], f=emit_pv: f(*r))
                else:
                    def flush_pvs(rem=rem, emit_pv=emit_pv):
                        for item in rem:
                            emit_pv(*item)

                    def normalize(out_pss=out_pss, q0=q0):
                        norm_cols(out_pss, q0, 0, QSB)

                    tails = (flush_pvs, normalize)

                if pending_tail is not None:
                    # n_kb too small to have reached the injection points
                    pending_tail[0]()
                    pending_tail[1]()
                pending_tail = tails
                done.append((b, qsb))
                if pos in PROJ_AFTER_LOOP:
                    emit_proj(TASK_ORDER[PROJ_AFTER_LOOP[pos]])

            # drain the final task's tail
            pending_tail[0]()
            pending_tail[1]()

            # flush the last deferred projection group
            emit_proj(TASK_ORDER[-1], final=True)
    nc.compile()
    _CACHED_NC = nc
    return nc


def make_in_maps(x, w_qkv):
    """Host-side prep: transpose/cast/slice the full inputs per core."""
    xt = np.asarray(x, dtype=np.float32).reshape(T, D).T  # [D, T]
    xt = np.ascontiguousarray(
        xt.reshape(NKC, 128, T).transpose(1, 0, 2)).astype(NP_BF16)
    w_qkv = np.asarray(w_qkv, dtype=np.float32)
    scale = DH ** -0.5
    k_idx = np.arange(128)
    tri = (k_idx[:, None] <= k_idx[None, :])
    tri_nc = tri | ((k_idx[:, None] < NONCAUSAL) & (k_idx[None, :] < NONCAUSAL))
    mpack = np.zeros((128, 320), dtype=NP_BF16)
    mpack[:, 0:128] = tri.astype(NP_BF16)
    mpack[:, 128:256] = tri_nc.astype(NP_BF16)
    mpack[0, 256:256 + DH] = 1.0

    in_maps = []
    for c in range(N_CORES):
        h0 = c * HPC
        cols = slice(h0 * DH, (h0 + HPC) * DH)
        wq = w_qkv[:, 0 * HEADS * DH:][:, cols] * scale
        wk = w_qkv[:, 1 * HEADS * DH:][:, cols]
        wv = w_qkv[:, 2 * HEADS * DH:][:, cols]
        # pack as [128, 3, NKC, IPC]: partition = d % 128, chunk = d // 128
        wqkv_pack = np.ascontiguousarray(
            np.stack([wq, wk, wv], axis=0)        # [3, D, IPC]
            .reshape(3, NKC, 128, IPC)
            .transpose(2, 0, 1, 3)).astype(NP_BF16)
        in_maps.append({
            "xt": xt,
            "wqkv": wqkv_pack,
            "wo": None,  # filled below
            "mpack": mpack,
        })
    return in_maps


def run(x, mask, w_qkv, w_out, b_out, trace=False, **spmd_kwargs):
    nc = build_nc()
    in_maps = make_in_maps(x, w_qkv)
    w_out = np.asarray(w_out, dtype=np.float32)
    for c in range(N_CORES):
        in_maps[c]["wo"] = np.ascontiguousarray(
            w_out[c * IPC:(c + 1) * IPC, :]).astype(NP_BF16)
    res = run_bass_kernel_spmd(
        nc, in_maps, core_ids=list(range(N_CORES)), trace=trace, **spmd_kwargs)
    partial = np.zeros((T, D), dtype=np.float32)
    for c in range(N_CORES):
        partial += res.results[c]["out"].astype(np.float32)
    partial += np.asarray(b_out, dtype=np.float32)[None, :]
    return partial.reshape(B, N, D), res


def _axon_reset():
    """Recover a wedged axon-tunneled device (best effort)."""
    try:
        import ctypes
        import jax
        jax.devices()
        lib = ctypes.CDLL("/opt/axon/libaxon_pjrt.so")
        lib.axon_reset.restype = ctypes.c_int64
        lib.axon_reset()
    except Exception:
        pass


def kernel(x, mask, w_qkv, w_out, b_out):
    try:
        out, _ = run(x, mask, w_qkv, w_out, b_out, trace=False)
    except Exception:
        _axon_reset()
        out, _ = run(x, mask, w_qkv, w_out, b_out, trace=False)
    return out
